# revision 4
# baseline (speedup 1.0000x reference)
"""Self-contained Trainium2 Bass kernel for the LSS voxel-pooling problem
(nn_DSFusionv2_28819230556604).

kernel(**inputs) takes the FULL unsharded inputs (numpy) and returns the
FULL [B, C, NZ, NY, NX] float32 output.

Strategy (8 NeuronCores, data-parallel over flattened kept rows):
  The camera geometry makes voxel indices separable per (b,n,d) "slice":
  the BEV cell (ix,iy) depends only on (n,d,w) and the z in-bounds mask
  only on (n,d,h).  The host mirrors the reference's float32 index math
  exactly and keeps only the rows (b,n,d,h) that are in bounds (~88%);
  whole slices whose ego-x falls outside the grid are dropped.

  Kept rows are packed densely, 128 per PE group, split contiguously
  across the 8 cores (a slice crossing a group/core boundary simply
  yields two partial sums - the host merge adds them back together).

  Device pipeline per core (pure stage-A, memory-roofline bound):
    all G x-group DMAs + the tiny row->slice one-hot Z are issued up
    front on three queues; for each group one [128 -> S_g, 3520] mask
    matmul per 512-col PSUM bank reduces the rows onto slice partials;
    each group's stripe is copied to bf16 and streamed out immediately,
    so the output DMA overlaps the remaining groups.

  Host merges the per-(slice-part, w) rows into the BEV canvas with one
  vectorized scatter-add (duplicate cells within a slice, slice parts,
  and cross-core parts all collapse there).
"""
import os
import numpy as np
import ml_dtypes

# ---- problem constants (hardcoded from the reference config) ----
B, N, D, FH, FW, C = 2, 6, 48, 16, 44, 80
OGH, OGW = 256, 704
D_MIN, D_MAX = 2.0, 58.0
NX, NY, NZ = 256, 256, 1
LOWER = np.array([-51.2, -51.2, -10.0], np.float32)
DX = np.array([0.4, 0.4, 20.0], np.float32)

NCORE = 8
WC = FW * C                       # 3520


def _frustum():
    ds = D_MIN + (D_MAX - D_MIN) / D * np.arange(D, dtype=np.float32)
    ds = np.broadcast_to(ds[:, None, None], (D, FH, FW))
    xs = np.broadcast_to(np.linspace(0, OGW - 1, FW, dtype=np.float32)[None, None, :], (D, FH, FW))
    ys = np.broadcast_to(np.linspace(0, OGH - 1, FH, dtype=np.float32)[None, :, None], (D, FH, FW))
    return np.stack([xs, ys, ds], -1)


def _geometry_indices(rots, trans, intrins, post_rots, post_trans):
    """Voxel indices, bit-matching the reference's float32 op sequence."""
    frustum = _frustum()
    pts = frustum[None, None] - post_trans[:, :, None, None, None, :]
    inv_post = np.linalg.inv(post_rots).astype(np.float32)
    pts = np.einsum('bnij,bndhwj->bndhwi', inv_post, pts).astype(np.float32)
    pts = np.concatenate([pts[..., :2] * pts[..., 2:3], pts[..., 2:3]], axis=-1)
    combine = np.einsum('bnij,bnjk->bnik', rots,
                        np.linalg.inv(intrins).astype(np.float32)).astype(np.float32)
    pts = np.einsum('bnij,bndhwj->bndhwi', combine, pts).astype(np.float32)
    geom = (pts + trans[:, :, None, None, None, :]).astype(np.float32)
    gi = ((geom - LOWER) / DX).astype(np.int32)
    kept = ((gi[..., 0] >= 0) & (gi[..., 0] < NX) &
            (gi[..., 1] >= 0) & (gi[..., 1] < NY) &
            (gi[..., 2] >= 0) & (gi[..., 2] < NZ))
    return gi, kept


def _build_slices(gi, kept):
    """Per-(b,n,d) slice descriptors: kept h rows + per-w BEV cell ids."""
    slices = []
    for b in range(B):
        for n in range(N):
            for d in range(D):
                g = gi[b, n, d]
                k = kept[b, n, d]
                if not (g[..., 0] == g[0:1, :, 0]).all() or not (g[..., 1] == g[0:1, :, 1]).all():
                    raise RuntimeError("structure violation: gi_x/gi_y vary with h")
                zok = (g[:, :, 2] >= 0) & (g[:, :, 2] < NZ)
                if not (zok == zok[:, 0:1]).all():
                    raise RuntimeError("structure violation: z-ok varies with w")
                xyok = ((g[0, :, 0] >= 0) & (g[0, :, 0] < NX) &
                        (g[0, :, 1] >= 0) & (g[0, :, 1] < NY))
                if not (k == (zok[:, 0:1] & xyok[None, :])).all():
                    raise RuntimeError("structure violation: kept not separable")
                if not xyok.any():
                    continue
                hs = np.nonzero(zok[:, 0])[0]
                if hs.size == 0:
                    continue
                cells = np.where(xyok, g[0, :, 1].astype(np.int64) * NX + g[0, :, 0], -1)
                slices.append((b, n, d, hs, cells))
    return slices


def _build_nc(G, Sg, offs, TOT, Smax):
    import concourse.bacc as bacc
    import concourse.mybir as mybir
    import concourse.tile as tile
    F32 = mybir.dt.float32
    BF16 = mybir.dt.bfloat16

    nc = bacc.Bacc(None, target_bir_lowering=True)
    x_d = nc.dram_tensor("x", [G * 128, WC], BF16, kind="ExternalInput")
    z_d = nc.dram_tensor("z", [128, G, Smax], BF16, kind="ExternalInput")
    out_d = nc.dram_tensor("out", [TOT, WC], BF16, kind="ExternalOutput")

    with tile.TileContext(nc) as tc:
        with (
            tc.tile_pool(name="sbuf", bufs=1) as pool,
            tc.tile_pool(name="xin", bufs=G) as xpool,
            tc.tile_pool(name="psum", bufs=1, space="PSUM") as psum,
        ):
            ztile = pool.tile([128, G, Smax], BF16)
            nc.scalar.dma_start(ztile[:], z_d[:])
            xgs = []
            for g in range(G):
                xg = xpool.tile([128, WC], BF16)
                eng = nc.sync if g % 2 == 0 else nc.gpsimd
                eng.dma_start(xg[:], x_d[128 * g:128 * (g + 1), :])
                xgs.append(xg)

            psumT = psum.tile([128, WC], F32, tag="ps")
            outbf = pool.tile([128, WC], BF16)
            for g in range(G):
                s, o = Sg[g], offs[g]
                q = (g % 3) * 32          # matmul out base partition must be 0/32/64
                for c0 in range(0, WC, 512):
                    w = min(512, WC - c0)
                    nc.tensor.matmul(
                        psumT[q:q + s, c0:c0 + w],
                        ztile[:, g, 0:s], xgs[g][:, c0:c0 + w],
                        start=True, stop=True, skip_group_check=True,
                    )
                nc.vector.tensor_copy(outbf[q:q + s, :], psumT[q:q + s, :])
                nc.scalar.dma_start(out_d[o:o + s, :], outbf[q:q + s, :])
    nc.compile()
    return nc


_NC_CACHE = {}
_LAST_EXEC_NS = None


def kernel(x, rots, trans, intrins, post_rots, post_trans):
    global _LAST_EXEC_NS
    x = np.asarray(x)
    rots = np.asarray(rots, np.float32)
    trans = np.asarray(trans, np.float32)
    intrins = np.asarray(intrins, np.float32)
    post_rots = np.asarray(post_rots, np.float32)
    post_trans = np.asarray(post_trans, np.float32)

    gi, kept = _geometry_indices(rots, trans, intrins, post_rots, post_trans)
    slices = _build_slices(gi, kept)

    # global dense row list -> 8 contiguous core chunks -> groups of 128
    row_si = np.concatenate([np.full(len(s[3]), i, np.int64)
                             for i, s in enumerate(slices)])
    row_b = np.concatenate([np.full(len(s[3]), s[0], np.int64) for s in slices])
    row_n = np.concatenate([np.full(len(s[3]), s[1], np.int64) for s in slices])
    row_d = np.concatenate([np.full(len(s[3]), s[2], np.int64) for s in slices])
    row_h = np.concatenate([s[3].astype(np.int64) for s in slices])
    R = len(row_si)
    bounds = [R * c // NCORE for c in range(NCORE + 1)]
    G = -(-max(bounds[c + 1] - bounds[c] for c in range(NCORE)) // 128)

    # per-core, per-group local slice tables; Sg = max local slices per group
    core_groups = []           # [core][g] -> list of global slice ids
    for c in range(NCORE):
        lo, hi = bounds[c], bounds[c + 1]
        groups = []
        for g in range(G):
            seg = row_si[lo + g * 128: min(lo + (g + 1) * 128, hi)]
            ids = list(dict.fromkeys(seg.tolist()))
            groups.append(ids)
        core_groups.append(groups)
    Sg = [max(len(core_groups[c][g]) for c in range(NCORE)) for g in range(G)]
    offs = np.concatenate([[0], np.cumsum(Sg)]).astype(int)
    TOT = int(offs[-1])
    Smax = max(Sg)
    if TOT > 128:
        raise RuntimeError(f"psum overflow: {TOT} slice rows")
    if Smax > 32:
        raise RuntimeError(f"psum stripe overflow: {Smax} slices in a group")

    inmaps = []
    for c in range(NCORE):
        lo, hi = bounds[c], bounds[c + 1]
        xc = np.zeros((G * 128, WC), ml_dtypes.bfloat16)
        xc[0:hi - lo] = (x[row_b[lo:hi], row_n[lo:hi], row_d[lo:hi], row_h[lo:hi]]
                         .reshape(hi - lo, WC).astype(ml_dtypes.bfloat16))
        z = np.zeros((128, G, Smax), ml_dtypes.bfloat16)
        for g in range(G):
            seg = row_si[lo + g * 128: min(lo + (g + 1) * 128, hi)]
            lut = {si: j for j, si in enumerate(core_groups[c][g])}
            for p, si in enumerate(seg.tolist()):
                z[p, g, lut[si]] = 1.0
        inmaps.append({"x": xc, "z": z})

    key = (G, tuple(Sg))
    if key not in _NC_CACHE:
        _NC_CACHE[key] = _build_nc(G, Sg, offs, TOT, Smax)
    from concourse.bass_utils import run_bass_kernel_spmd
    trace = bool(int(os.environ.get("LSS_TRACE", "0")))
    if not trace:
        # the NTFF trace path needs antenv.axon_hooks, absent in this image;
        # make sure a global BASS_TRACE=1 can't route us there
        os.environ["BASS_NEVER_TRACE"] = "1"
    res = run_bass_kernel_spmd(_NC_CACHE[key], inmaps, core_ids=list(range(NCORE)),
                               trace=trace)
    _LAST_EXEC_NS = res.exec_time_ns

    # host merge: per-(slice-part, w) rows -> BEV canvas scatter-add
    canvas = np.zeros((B, NY * NX, C), np.float64)
    per_b_cells = [[] for _ in range(B)]
    per_b_vals = [[] for _ in range(B)]
    for c, r in enumerate(res.results):
        dev = np.asarray(r["out"]).astype(np.float32)   # [TOT, WC]
        for g in range(G):
            for j, si in enumerate(core_groups[c][g]):
                b, _, _, _, cells = slices[si]
                m = cells >= 0
                vals = dev[offs[g] + j].reshape(FW, C)
                per_b_cells[b].append(cells[m])
                per_b_vals[b].append(vals[m])
    for b in range(B):
        if per_b_cells[b]:
            np.add.at(canvas[b], np.concatenate(per_b_cells[b]),
                      np.concatenate(per_b_vals[b]).astype(np.float64))
    out = (canvas.reshape(B, NY, NX, C).transpose(0, 3, 1, 2)[:, :, None]
           .astype(np.float32))
    return np.ascontiguousarray(out.reshape(B, C, NZ, NY, NX))


# revision 23
# speedup vs baseline: 1.9157x; 1.9157x over previous
"""Self-contained Trainium2 Bass kernel for the LSS voxel-pooling problem
(nn_DSFusionv2_28819230556604).

kernel(**inputs) takes the FULL unsharded inputs (numpy) and returns the
FULL [B, C, NZ, NY, NX] float32 output.

Strategy (8 NeuronCores, data-parallel over flattened kept rows):
  The camera geometry makes voxel indices separable per (b,n,d) "slice":
  the BEV cell (ix,iy) depends only on (n,d,w) and the z in-bounds mask
  only on (n,d,h).  The host mirrors the reference's float32 index math
  exactly and keeps only the rows (b,n,d,h) that are in bounds (~88%);
  whole slices whose ego-x falls outside the grid are dropped.

  Kept rows are packed densely, 128 per PE group, split contiguously
  across the 8 cores (a slice crossing a group/core boundary simply
  yields two partial sums - the host merge adds them back together).

  Device pipeline per core (pure stage-A, memory-roofline bound):
    all G x-group DMAs + the tiny row->slice one-hot Z are issued up
    front on three queues; for each group one [128 -> S_g, 3520] mask
    matmul per 512-col PSUM bank reduces the rows onto slice partials;
    each group's stripe is copied to bf16 and streamed out immediately,
    so the output DMA overlaps the remaining groups.

  Host merges the per-(slice-part, w) rows into the BEV canvas with one
  vectorized scatter-add (duplicate cells within a slice, slice parts,
  and cross-core parts all collapse there).
"""
import os
import numpy as np
import ml_dtypes

# ---- problem constants (hardcoded from the reference config) ----
B, N, D, FH, FW, C = 2, 6, 48, 16, 44, 80
OGH, OGW = 256, 704
D_MIN, D_MAX = 2.0, 58.0
NX, NY, NZ = 256, 256, 1
LOWER = np.array([-51.2, -51.2, -10.0], np.float32)
DX = np.array([0.4, 0.4, 20.0], np.float32)

NCORE = 8
WC = FW * C                       # 3520


def _frustum():
    ds = D_MIN + (D_MAX - D_MIN) / D * np.arange(D, dtype=np.float32)
    ds = np.broadcast_to(ds[:, None, None], (D, FH, FW))
    xs = np.broadcast_to(np.linspace(0, OGW - 1, FW, dtype=np.float32)[None, None, :], (D, FH, FW))
    ys = np.broadcast_to(np.linspace(0, OGH - 1, FH, dtype=np.float32)[None, :, None], (D, FH, FW))
    return np.stack([xs, ys, ds], -1)


def _geometry_indices(rots, trans, intrins, post_rots, post_trans):
    """Voxel indices, bit-matching the reference's float32 op sequence."""
    frustum = _frustum()
    pts = frustum[None, None] - post_trans[:, :, None, None, None, :]
    inv_post = np.linalg.inv(post_rots).astype(np.float32)
    pts = np.einsum('bnij,bndhwj->bndhwi', inv_post, pts).astype(np.float32)
    pts = np.concatenate([pts[..., :2] * pts[..., 2:3], pts[..., 2:3]], axis=-1)
    combine = np.einsum('bnij,bnjk->bnik', rots,
                        np.linalg.inv(intrins).astype(np.float32)).astype(np.float32)
    pts = np.einsum('bnij,bndhwj->bndhwi', combine, pts).astype(np.float32)
    geom = (pts + trans[:, :, None, None, None, :]).astype(np.float32)
    gi = ((geom - LOWER) / DX).astype(np.int32)
    kept = ((gi[..., 0] >= 0) & (gi[..., 0] < NX) &
            (gi[..., 1] >= 0) & (gi[..., 1] < NY) &
            (gi[..., 2] >= 0) & (gi[..., 2] < NZ))
    return gi, kept


def _build_slices(gi, kept):
    """Per-(b,n,d) slice descriptors: kept h rows + per-w BEV cell ids."""
    slices = []
    for b in range(B):
        for n in range(N):
            for d in range(D):
                g = gi[b, n, d]
                k = kept[b, n, d]
                if not (g[..., 0] == g[0:1, :, 0]).all() or not (g[..., 1] == g[0:1, :, 1]).all():
                    raise RuntimeError("structure violation: gi_x/gi_y vary with h")
                zok = (g[:, :, 2] >= 0) & (g[:, :, 2] < NZ)
                if not (zok == zok[:, 0:1]).all():
                    raise RuntimeError("structure violation: z-ok varies with w")
                xyok = ((g[0, :, 0] >= 0) & (g[0, :, 0] < NX) &
                        (g[0, :, 1] >= 0) & (g[0, :, 1] < NY))
                if not (k == (zok[:, 0:1] & xyok[None, :])).all():
                    raise RuntimeError("structure violation: kept not separable")
                if not xyok.any():
                    continue
                hs = np.nonzero(zok[:, 0])[0]
                if hs.size == 0:
                    continue
                cells = np.where(xyok, g[0, :, 1].astype(np.int64) * NX + g[0, :, 0], -1)
                slices.append((b, n, d, hs, cells))
    return slices


def _build_nc(G, TOT):
    TOTP = 128
    import concourse.bacc as bacc
    import concourse.mybir as mybir
    import concourse.tile as tile
    F32 = mybir.dt.float32
    BF16 = mybir.dt.bfloat16
    FP8 = mybir.dt.float8e4

    nc = bacc.Bacc(None, target_bir_lowering=True)
    x_d = nc.dram_tensor("x", [(G - 1) * 128, WC], BF16, kind="ExternalInput")
    x2_d = nc.dram_tensor("x2", [7, 128, 512], BF16, kind="ExternalInput")
    z_d = nc.dram_tensor("z", [128, G, TOT], FP8, kind="ExternalInput")
    out_d = nc.dram_tensor("out", [TOTP, WC], BF16, kind="ExternalOutput")

    with tile.TileContext(nc) as tc:
        with (
            tc.tile_pool(name="sbuf", bufs=1) as pool,
            tc.tile_pool(name="psum", bufs=1, space="PSUM") as psum,
        ):
            ztile = pool.tile([128, G, TOT], FP8)
            nc.scalar.dma_start(ztile[:], z_d[:])
            xgs = []
            for g in range(G - 1):
                xg = pool.tile([128, WC], BF16, name=f"xg{g}")
                nc.sync.dma_start(xg[:], x_d[128 * g:128 * (g + 1), :])
                xgs.append(xg)
            # last group ships chunk-major (contiguous per 512-col chunk) so
            # its matmuls and the drain chase the tail of the stream
            xlast = pool.tile([128, WC], BF16, name="xlast")
            for k, c0 in enumerate(range(0, WC, 512)):
                w = min(512, WC - c0)
                nc.sync.dma_start(xlast[:, c0:c0 + w], x2_d[k, :, 0:w])
            xgs.append(xlast)
            # one PSUM tile per 512-col bank (hw matmul N<=512) so dep
            # tracking stays per-bank and drain copies chase the last group
            psks = [psum.tile([128, 512], F32, tag=f"ps{k}", name=f"ps{k}") for k in range(7)]
            for g in range(G):
                for k, c0 in enumerate(range(0, WC, 512)):
                    w = min(512, WC - c0)
                    nc.tensor.matmul(
                        psks[k][0:TOT, 0:w],
                        ztile[:, g, 0:TOT], xgs[g][:, c0:c0 + w],
                        start=(g == 0), stop=(g == G - 1),
                        skip_group_check=True,
                    )
            # drain: DVE converts banks 0-3, ACT banks 4-6 (single-writer
            # staging tiles); writeback split three ways, each issued as soon
            # as its banks are converted
            outA = pool.tile([128, 4, 512], BF16)
            outB = pool.tile([128, 3, 512], BF16)
            for k in range(4):
                nc.vector.tensor_copy(outA[:, k, :], psks[k][:, 0:512])
            for k in range(4, 7):
                w = min(512, WC - 512 * k)
                nc.scalar.copy(outB[:, k - 4, 0:w], psks[k][:, 0:w])
            nc.sync.dma_start(
                out_d[:, 0:1024],
                outA[:, 0:2, :].rearrange("p a b -> p (a b)"))
            nc.sync.dma_start(
                out_d[:, 1024:2048],
                outA[:, 2:4, :].rearrange("p a b -> p (a b)"))
            nc.scalar.dma_start(
                out_d[:, 2048:WC],
                outB[:].rearrange("p a b -> p (a b)")[:, 0:WC - 2048])
    nc.compile()
    return nc


_NC_CACHE = {}
_LAST_EXEC_NS = None


def kernel(x, rots, trans, intrins, post_rots, post_trans):
    global _LAST_EXEC_NS
    x = np.asarray(x)
    rots = np.asarray(rots, np.float32)
    trans = np.asarray(trans, np.float32)
    intrins = np.asarray(intrins, np.float32)
    post_rots = np.asarray(post_rots, np.float32)
    post_trans = np.asarray(post_trans, np.float32)

    gi, kept = _geometry_indices(rots, trans, intrins, post_rots, post_trans)
    slices = _build_slices(gi, kept)

    # global dense row list -> 8 contiguous core chunks -> groups of 128
    row_si = np.concatenate([np.full(len(s[3]), i, np.int64)
                             for i, s in enumerate(slices)])
    row_b = np.concatenate([np.full(len(s[3]), s[0], np.int64) for s in slices])
    row_n = np.concatenate([np.full(len(s[3]), s[1], np.int64) for s in slices])
    row_d = np.concatenate([np.full(len(s[3]), s[2], np.int64) for s in slices])
    row_h = np.concatenate([s[3].astype(np.int64) for s in slices])
    R = len(row_si)
    bounds = [R * c // NCORE for c in range(NCORE + 1)]
    G = -(-max(bounds[c + 1] - bounds[c] for c in range(NCORE)) // 128)

    # per-core global slice tables (slices may span groups/cores; host merges)
    core_slices = []           # [core] -> list of global slice ids, in order
    for c in range(NCORE):
        lo, hi = bounds[c], bounds[c + 1]
        core_slices.append(list(dict.fromkeys(row_si[lo:hi].tolist())))
    TOT = max(len(cs) for cs in core_slices)
    TOTP = 128
    if TOT > 128:
        raise RuntimeError(f"psum overflow: {TOT} slice rows")

    inmaps = []
    for c in range(NCORE):
        lo, hi = bounds[c], bounds[c + 1]
        xc = np.zeros((G * 128, WC), ml_dtypes.bfloat16)
        xc[0:hi - lo] = (x[row_b[lo:hi], row_n[lo:hi], row_d[lo:hi], row_h[lo:hi]]
                         .reshape(hi - lo, WC).astype(ml_dtypes.bfloat16))
        x2 = np.zeros((7, 128, 512), ml_dtypes.bfloat16)
        lastg = xc[(G - 1) * 128:G * 128]
        for k in range(7):
            c0 = 512 * k
            w = min(512, WC - c0)
            x2[k, :, 0:w] = lastg[:, c0:c0 + w]
        lut = {si: j for j, si in enumerate(core_slices[c])}
        z = np.zeros((128, G, TOT), ml_dtypes.float8_e4m3)
        sis = row_si[lo:hi]
        for r in range(hi - lo):
            z[r % 128, r // 128, lut[sis[r]]] = 1.0
        inmaps.append({"x": np.ascontiguousarray(xc[0:(G - 1) * 128]), "x2": x2, "z": z})

    key = (G, TOT)
    if key not in _NC_CACHE:
        _NC_CACHE[key] = _build_nc(G, TOT)
    from concourse.bass_utils import run_bass_kernel_spmd
    trace = bool(int(os.environ.get("LSS_TRACE", "0")))
    if not trace:
        # the NTFF trace path needs antenv.axon_hooks, absent in this image;
        # make sure a global BASS_TRACE=1 can't route us there
        os.environ["BASS_NEVER_TRACE"] = "1"
    res = run_bass_kernel_spmd(_NC_CACHE[key], inmaps, core_ids=list(range(NCORE)),
                               trace=trace)
    _LAST_EXEC_NS = res.exec_time_ns
    globals()['_LAST_RES'] = res

    # host merge: per-(slice-part, w) rows -> BEV canvas scatter-add
    canvas = np.zeros((B, NY * NX, C), np.float64)
    per_b_cells = [[] for _ in range(B)]
    per_b_vals = [[] for _ in range(B)]
    for c, r in enumerate(res.results):
        dev = np.asarray(r["out"]).astype(np.float32)   # [TOT, WC]
        for j, si in enumerate(core_slices[c]):
            b, _, _, _, cells = slices[si]
            m = cells >= 0
            vals = dev[j].reshape(FW, C)
            per_b_cells[b].append(cells[m])
            per_b_vals[b].append(vals[m])
    for b in range(B):
        if per_b_cells[b]:
            np.add.at(canvas[b], np.concatenate(per_b_cells[b]),
                      np.concatenate(per_b_vals[b]).astype(np.float64))
    out = (canvas.reshape(B, NY, NX, C).transpose(0, 3, 1, 2)[:, :, None]
           .astype(np.float32))
    return np.ascontiguousarray(out.reshape(B, C, NZ, NY, NX))
